# revision 2
# baseline (speedup 1.0000x reference)
"""Multi-head attention (B=2, H=16, S=2048, D=64) on 8 trn2 NeuronCores.

Sharding: the 32 (b, h) head-units are split 4-per-core (head/data parallel,
no cross-core comms).  Per core, for each head:

  scoresT[k, q] = sum_d K[k, d] Q[q, d]            (PE, contract=64, row-packed 2x)
  pT[k, q]      = masked-exp(scoresT / 8)          (split across ACT and DVE)
  OT'[m, q]     = sum_k V'[k, m] pT[k, q]          (PE, V' = [V | ones] so row 64
                                                    of OT' is the softmax denom Z)
  out[q, d]     = OT'[d, q] / OT'[64, q]           (host-side: O(S*D) divide +
                                                    transpose while unsharding)

v2 restructure: query blocks are processed in PAIRS (sa, sb = 2p, 2p+1) per
head, chunk-major.  Both QK matmuls of a chunk share the same stationary
K^T slice, and both PV matmuls of a chunk share the same V' slice, so every
LDWEIGHTS is amortized over two N=512 matmuls (the k-chunk weights never
depended on the query block; the old per-(sqb,h) loop reloaded them anyway).
The exp ops span both query blocks of one chunk (FD=1024), so the
elementwise cost per element is unchanged while the PE drops its exposed
weight-load time.

The exp+mask stage is split across the two PSUM-capable elementwise engines:

  * ACT chunks (0..NACT-1): nc.scalar.activation Exp at 1 elem/cyc @1.2 GHz,
    then a DVE fp16 tensor_tensor multiply with the 0/1 keep mask (2x_1P).
  * DVE chunks (NACT..15): one fused scalar_tensor_tensor computes
    uint16 t = round(raw*184.665 + B) saturating at [0, 65535]; the uint16
    bit pattern *is* fp16 exp(raw/8) (Schraudolph bit trick, ~1% softmax
    error) and B encodes the mask additively (keep: 15328, masked: -60000
    -> t saturates to 0 -> fp16 +0.0).  One 1-elem/cyc DVE pass replaces
    the exp AND the mask multiply.

Working in the transposed-score layout means softmax needs no reductions at
all (Z rides along in the PV matmul) and no S x S transposes anywhere.

Host-side: Q/K pre-transposed per head as [64, S]; V chunk-interleaved fp16
with a ones column appended; the shared mask transposed, chunk-major with
the two query blocks of a pair interleaved per chunk, ACT chunks encoded as
0/1 fp16 and DVE chunks as the Schraudolph bias tensor.
"""

import numpy as np

import concourse.bass as bass  # noqa: F401  (engine types resolve through nc)
import concourse.mybir as mybir
import concourse.tile as tile
from concourse import bacc
from concourse.bass_utils import run_bass_kernel_spmd

B, H, S, D = 2, 16, 2048, 64
N_CORES = 8
HPC = (B * H) // N_CORES  # heads per core

SQ = 512        # query-block width (one fp32 PSUM bank)
CK = 128        # key-chunk height (PSUM partition dim)
PW = 2 * SQ     # pair width: both query blocks of one chunk, side by side
QK_BUFS = 3     # qk PSUM slots (2 banks each) + 2 o_ps accumulators = 8 banks
NACT = 12       # chunks 0..NACT-1 -> ACT exp path, rest -> DVE Schraudolph
VW = D + 2      # V' width: 64 V columns + ones column + pad (66)

A_SCH = 184.664964          # 1024 * log2(e) / 8
B_KEEP = 15328.0            # 15*1024 + 1024*corr, corr=-0.03125 (minimax-ish)
B_MASK = -60000.0           # saturates uint16 convert to 0 -> fp16 +0.0

f32 = mybir.dt.float32
f16 = mybir.dt.float16
u16 = mybir.dt.uint16
FT = mybir.ActivationFunctionType


def build_nc(hpc=HPC, s=S, loop_n=None, ablate=(), nact=NACT,
             tail_engine="act", loop_stagger=False):
    """Build the per-core Bass program (identical on all 8 cores).

    loop_n: if set, wrap the whole body in an on-device For_i loop that
    recomputes the same output loop_n times — a perf-measurement rig that
    lets wall-clock deltas between two loop_n values cancel host/RPC
    overheads (this container has no NTFF profile path).

    ablate: perf-debug only — subset of {"qk", "act", "mask", "pv", "tail"}
    to skip emitting, isolating per-engine throughput on HW. Output is
    garbage when non-empty.
    """
    nsq = s // SQ
    nck = s // CK
    npair = nsq // 2
    ablate = set(ablate)

    nc = bacc.Bacc("TRN2", target_bir_lowering=False, debug=False)

    qt_d = nc.dram_tensor("qt", [hpc, D, s], f16, kind="ExternalInput")
    kt_d = nc.dram_tensor("kt", [hpc, D, s], f16, kind="ExternalInput")
    vp_d = nc.dram_tensor("vp", [hpc, CK, nck * VW], f16, kind="ExternalInput")
    mk_d = nc.dram_tensor("mk", [npair, CK, nck * PW], f16, kind="ExternalInput")
    o_d = nc.dram_tensor("o", [hpc, nsq, VW, SQ], f16, kind="ExternalOutput")

    with tile.TileContext(nc) as tc:
        if ablate:
            tc.race_detector_enabled = False
        with (
            tc.tile_pool(name="heads", bufs=hpc) as head_pool,
            tc.tile_pool(name="mask", bufs=npair) as mask_pool,
            # bufs=3: pair j's exp writes while pair j-1's interleaved PV
            # matmuls still read; a third buffer decouples the two.
            tc.tile_pool(name="pt", bufs=3) as pt_pool,
            tc.tile_pool(name="tail", bufs=2) as tail_pool,
            tc.tile_pool(name="qk_ps", bufs=QK_BUFS, space="PSUM") as qk_pool,
            tc.tile_pool(name="o_ps", bufs=2, space="PSUM") as o_pool,
        ):
            qt_t, kt_t, vp_t = [], [], []
            for h in range(hpc):
                q_t = head_pool.tile([128, s], f16, name=f"qt_sb{h}", tag="qt")
                k_t = head_pool.tile([128, s], f16, name=f"kt_sb{h}", tag="kt")
                v_t = head_pool.tile([CK, nck * VW], f16, name=f"vp_sb{h}", tag="vp")
                # Q^T/K^T live duplicated in both partition halves so the two
                # row-packed K=64 matmuls can run concurrently on the PE.
                nc.sync.dma_start(out=q_t[0:D, :], in_=qt_d[h, :, :])
                nc.sync.dma_start(out=q_t[D:128, :], in_=qt_d[h, :, :])
                nc.sync.dma_start(out=k_t[0:D, :], in_=kt_d[h, :, :])
                nc.sync.dma_start(out=k_t[D:128, :], in_=kt_d[h, :, :])
                nc.sync.dma_start(out=v_t[:, :], in_=vp_d[h, :, :])
                qt_t.append(q_t)
                kt_t.append(k_t)
                vp_t.append(v_t)

            # The whole mask fits in SBUF — load it once, outside any
            # measurement loop (saves 8MB of DMA per pass).
            mk_t = {}     # p -> mask tile [128, nck*PW] (chunk-major columns)
            for p in range(npair):
                mk = mask_pool.tile([CK, nck * PW], f16, name=f"mk_sb{p}",
                                    tag="mk")
                nc.sync.dma_start(out=mk[:, :], in_=mk_d[p, :, :])
                mk_t[p] = mk

            pt_t = {}     # pair-idx -> p^T tile [128, nck*PW] fp16
            o_ps = {}     # (pair-idx, u) -> PSUM accumulator [VW, SQ]

            def emit_qk_chunk(j, pairs, c, kind):
                """One chunk's two QK matmuls (same K^T weights, query blocks
                sa and sb) + exp (ACT) or fused Schraudolph (DVE)."""
                h, p = pairs[j]
                qk = None
                if "qk" not in ablate:
                    qk = qk_pool.tile([128, PW], f32, name=f"qk_{j}_{c}",
                                      tag="qk")
                    bp = 64 * (c % 2)  # row-half alternates with chunk parity
                    # so consecutive chunks' weight loads pull ahead
                    for u in range(2):
                        sqb = 2 * p + u
                        nc.tensor.matmul(
                            qk[:, u * SQ:(u + 1) * SQ],
                            lhsT=kt_t[h][bp:bp + D, c * CK:(c + 1) * CK],
                            rhs=qt_t[h][bp:bp + D, sqb * SQ:(sqb + 1) * SQ],
                            start=True,
                            stop=True,
                            tile_position=(bp, 0),
                        )
                pt = pt_t[j]
                if "act" in ablate or pt is None:
                    return
                lo = c * PW
                hi = (c + 1) * PW
                act_in = qk[:, :] if qk is not None else mk_t[p][:, lo:hi]
                if kind == "act":
                    nc.scalar.activation(pt[:, lo:hi], act_in, FT.Exp,
                                         scale=0.125)
                else:
                    nc.vector.scalar_tensor_tensor(
                        pt.bitcast(u16)[:, lo:hi], act_in, A_SCH,
                        mk_t[p][:, lo:hi],
                        op0=mybir.AluOpType.mult, op1=mybir.AluOpType.add,
                    )

            def emit_mask(j, pairs, clo, chi):
                """Apply the 0/1 keep-mask to ACT-path chunk cols [clo, chi)
                of p^T in one fp16 2x-mode DVE pass."""
                if "mask" in ablate:
                    return
                h, p = pairs[j]
                clo, chi = min(clo, nact), min(chi, nact)
                if clo >= chi:
                    return
                pt = pt_t[j]
                lo, hi = clo * PW, chi * PW
                nc.vector.tensor_tensor(
                    pt[:, lo:hi], pt[:, lo:hi], mk_t[p][:, lo:hi],
                    op=mybir.AluOpType.mult,
                )

            def emit_pv(j, pairs, clo, chi):
                """PV matmuls for chunks [clo, chi): per chunk, both query
                blocks' matmuls share the V' weights."""
                if "pv" in ablate:
                    return
                pt = pt_t[j]
                if pt is None:
                    pt = mk_t[pairs[j][1]]  # stand-in for PE-only ablations
                h, p = pairs[j]
                for c in range(clo, chi):
                    for u in range(2):
                        nc.tensor.matmul(
                            o_ps[(j, u)][:, :],
                            lhsT=vp_t[h][:, c * VW:c * VW + VW],
                            rhs=pt[:, c * PW + u * SQ:c * PW + (u + 1) * SQ],
                            start=(c == 0),
                            stop=(c == nck - 1),
                        )

            def emit_tail(j, pairs):
                """Evacuate O^T' (unnormalized + Z row) as fp16 and store."""
                if "tail" in ablate or "pv" in ablate:
                    return
                h, p = pairs[j]
                for u in range(2):
                    sqb = 2 * p + u
                    ot = tail_pool.tile([VW, SQ], f16, name=f"ot_{j}_{u}",
                                        tag="ot")
                    if tail_engine == "act":
                        nc.scalar.copy(ot[:, :], o_ps[(j, u)][:, :])
                    else:
                        nc.vector.tensor_copy(ot[:, :], o_ps[(j, u)][:, :])
                    nc.sync.dma_start(out=o_d[h, sqb, :, :], in_=ot[:, :])

            def emit_alloc(j):
                if not ({"act", "mask"} <= ablate):
                    pt_t[j] = pt_pool.tile(
                        [128, nck * PW], f16, name=f"pt_{j}", tag="pt")
                else:
                    pt_t[j] = None
                if "pv" not in ablate:
                    for u in range(2):
                        o_ps[(j, u)] = o_pool.tile(
                            [VW, SQ], f32, name=f"ops_{j}_{u}", tag="ops")

            def emit_pair(j, pairs):
                """Chunk-major emission for pair j, interleaved with pair
                j-1's PV blocks so the in-order PE queue always has PV work
                in hand while an exp it depends on (via PSUM-slot reuse) is
                still running on ACT/DVE."""
                emit_alloc(j)
                prev = j - 1 if j >= 1 else None
                if nck != 16:   # small-s debug builds: simple sequential
                    for c in range(nck):
                        emit_qk_chunk(j, pairs, c, "act" if c < nact else "dve")
                    emit_mask(j, pairs, 0, nck)
                    if prev is not None:
                        emit_drain(prev, pairs)
                    return
                acts = list(range(nact))
                dves = list(range(nact, nck))
                for c in acts[:6]:
                    emit_qk_chunk(j, pairs, c, "act")
                if prev is not None:
                    emit_pv(prev, pairs, 0, 8)
                for c in dves:
                    emit_qk_chunk(j, pairs, c, "dve")
                for c in acts[6:8]:
                    emit_qk_chunk(j, pairs, c, "act")
                if prev is not None:
                    emit_pv(prev, pairs, 8, nck)
                    emit_tail(prev, pairs)
                for c in acts[8:]:
                    emit_qk_chunk(j, pairs, c, "act")
                # Masks trail: they keep DVE busy through the next pair's QK
                # phase and nothing reads pt chunks 0..nact-1 until the PV
                # one pair later.
                for lo in range(0, nact, 4):
                    emit_mask(j, pairs, lo, min(lo + 4, nact))

            def emit_drain(j, pairs):
                emit_pv(j, pairs, 0, nck)
                emit_tail(j, pairs)

            def emit_all():
                pairs = [(h, p) for h in range(hpc) for p in range(npair)]
                for j in range(len(pairs)):
                    emit_pair(j, pairs)
                if pairs:
                    emit_drain(len(pairs) - 1, pairs)

            if loop_n is None:
                emit_all()
            else:
                hints = (mybir.EngineType.PE, mybir.EngineType.Activation,
                         mybir.EngineType.DVE)
                with tc.For_i(0, loop_n, 1, hint_engines=hints,
                              staggered_reset=bool(loop_stagger)):
                    emit_all()

    nc.finalize()
    return nc


def shard_inputs(K, Q, V, mask, hpc=HPC, s=S, n_cores=N_CORES, nact=NACT):
    """Full inputs -> per-core in_maps with device-friendly host layouts."""
    nsq = s // SQ
    nck = s // CK
    npair = max(nsq // 2, 1)
    n_units = n_cores * hpc
    Kf = np.asarray(K, np.float32).reshape(n_units, s, D)
    Qf = np.asarray(Q, np.float32).reshape(n_units, s, D)
    Vf = np.asarray(V, np.float32).reshape(n_units, s, D)
    keepT = (~np.asarray(mask).reshape(s, s)).T  # [k, q], True = attend
    # ACT chunks: 0/1 multiplier.  DVE chunks: Schraudolph additive bias.
    nact = nact if nck == 16 else nck
    mk_f = keepT.astype(np.float16).reshape(nck, CK, s)
    dve = np.where(keepT.reshape(nck, CK, s)[nact:] > 0, np.float16(B_KEEP),
                   np.float16(B_MASK))
    mk_full = np.concatenate([mk_f[:nact], dve], axis=0)  # [nck, CK, s]
    # -> [npair, CK, nck * (2*SQ)]: chunk-major, the pair's two query
    # blocks (u = 0, 1) side by side within each chunk's column span.
    mk_host = np.ascontiguousarray(
        mk_full.reshape(nck, CK, npair, 2, SQ)
        .transpose(2, 1, 0, 3, 4)
        .reshape(npair, CK, nck * 2 * SQ)
    )
    in_maps = []
    for c in range(n_cores):
        sl = slice(c * hpc, (c + 1) * hpc)
        qt = np.ascontiguousarray(Qf[sl].transpose(0, 2, 1)).astype(np.float16)
        kt = np.ascontiguousarray(Kf[sl].transpose(0, 2, 1)).astype(np.float16)
        vp = np.zeros((hpc, s, VW), np.float16)
        vp[:, :, :D] = Vf[sl]
        vp[:, :, D] = 1.0
        vp = np.ascontiguousarray(
            vp.reshape(hpc, nck, CK, VW).transpose(0, 2, 1, 3)
            .reshape(hpc, CK, nck * VW)
        )
        in_maps.append({"qt": qt, "kt": kt, "vp": vp, "mk": mk_host})
    return in_maps


_NC_CACHE = {}


def _get_nc():
    if "nc" not in _NC_CACHE:
        _NC_CACHE["nc"] = build_nc()
    return _NC_CACHE["nc"]


def run_sharded(in_maps, trace=False, **kwargs):
    return run_bass_kernel_spmd(
        _get_nc(), in_maps, core_ids=list(range(N_CORES)), trace=trace, **kwargs
    )


def unshard_output(per_core_raw, hpc=HPC, s=S):
    """[hpc, nsq, VW, SQ] raw blocks per core -> [n*hpc, s, D] normalized.

    Row D of each block is the softmax denominator Z; dividing and
    transposing here is O(S*D) host work (same order as unsharding).
    """
    n = len(per_core_raw)
    out = np.empty((n * hpc, s, D), np.float32)
    for c, o in enumerate(per_core_raw):
        of = np.asarray(o, np.float32)              # raw blocks arrive fp16
        ot = of[:, :, :D, :] / of[:, :, D:D + 1, :]  # [hpc, nsq, D, SQ]
        out[c * hpc:(c + 1) * hpc] = (
            ot.transpose(0, 1, 3, 2).reshape(hpc, s, D))
    return out


def assemble_output(results):
    out = unshard_output([results[c]["o"] for c in range(N_CORES)])
    return out.reshape(B, H, S, D)


def kernel(K, Q, V, mask):
    in_maps = shard_inputs(K, Q, V, mask)
    res = run_sharded(in_maps)
    return assemble_output(res.results)


# revision 14
# speedup vs baseline: 1.2262x; 1.2262x over previous
"""Multi-head attention (B=2, H=16, S=2048, D=64) on 8 trn2 NeuronCores.

Sharding: the 32 (b, h) head-units are split 4-per-core (head/data parallel,
no cross-core comms).  Per core, for each head:

  scoresT[k, q] = sum_d K[k, d] Q[q, d]            (PE, contract=64, row-packed 2x)
  pT[k, q]      = masked-exp(scoresT / 8)          (split across ACT and DVE)
  OT'[m, q]     = sum_k V'[k, m] pT[k, q]          (PE, V' = [V | ones] so row 64
                                                    of OT' is the softmax denom Z)
  out[q, d]     = OT'[d, q] / OT'[64, q]           (host-side: O(S*D) divide +
                                                    transpose while unsharding)

v2 restructure: query blocks are processed in PAIRS (sa, sb = 2p, 2p+1) per
head, chunk-major.  Both QK matmuls of a chunk share the same stationary
K^T slice, and both PV matmuls of a chunk share the same V' slice, so every
LDWEIGHTS is amortized over two N=512 matmuls (the k-chunk weights never
depended on the query block; the old per-(sqb,h) loop reloaded them anyway).
The exp ops span both query blocks of one chunk (FD=1024), so the
elementwise cost per element is unchanged while the PE drops its exposed
weight-load time.

The exp+mask stage is split across the two PSUM-capable elementwise engines:

  * ACT chunks (0..NACT-1): nc.scalar.activation Exp at 1 elem/cyc @1.2 GHz,
    then a DVE fp16 tensor_tensor multiply with the 0/1 keep mask (2x_1P).
  * DVE chunks (NACT..15): one fused scalar_tensor_tensor computes
    uint16 t = round(raw*184.665 + B) saturating at [0, 65535]; the uint16
    bit pattern *is* fp16 exp(raw/8) (Schraudolph bit trick, ~1% softmax
    error) and B encodes the mask additively (keep: 15328, masked: -60000
    -> t saturates to 0 -> fp16 +0.0).  One 1-elem/cyc DVE pass replaces
    the exp AND the mask multiply.

Working in the transposed-score layout means softmax needs no reductions at
all (Z rides along in the PV matmul) and no S x S transposes anywhere.

Host-side: Q/K pre-transposed per head as [64, S]; V chunk-interleaved fp16
with a ones column appended; the shared mask transposed, chunk-major with
the two query blocks of a pair interleaved per chunk, ACT chunks encoded as
0/1 fp16 and DVE chunks as the Schraudolph bias tensor.
"""

import numpy as np

import concourse.bass as bass  # noqa: F401  (engine types resolve through nc)
import concourse.mybir as mybir
import concourse.tile as tile
from concourse import bacc
from concourse.bass_utils import run_bass_kernel_spmd

B, H, S, D = 2, 16, 2048, 64
N_CORES = 8
HPC = (B * H) // N_CORES  # heads per core

SQ = 512        # query-block width (one fp32 PSUM bank)
CK = 128        # key-chunk height (PSUM partition dim)
PW = 2 * SQ     # pair width: both query blocks of one chunk, side by side
QK_BUFS = 3     # qk PSUM slots (2 banks each) + 2 o_ps accumulators = 8 banks
NACT = 8        # chunks 0..NACT-1 -> ACT exp path, rest -> DVE Schraudolph
VW = D + 2      # V' width: 64 V columns + ones column + pad (66)

A_SCH = 184.664964          # 1024 * log2(e) / 8
B_KEEP = 15328.0            # 15*1024 + 1024*corr, corr=-0.03125 (minimax-ish)
B_MASK = -60000.0           # saturates uint16 convert to 0 -> fp16 +0.0

f32 = mybir.dt.float32
f16 = mybir.dt.float16
u16 = mybir.dt.uint16
FT = mybir.ActivationFunctionType


def build_nc(hpc=HPC, s=S, loop_n=None, ablate=(), nact=NACT,
             tail_engine="dve", loop_stagger=False, loop_body=1):
    """Build the per-core Bass program (identical on all 8 cores).

    loop_n: if set, wrap the whole body in an on-device For_i loop that
    recomputes the same output loop_n times — a perf-measurement rig that
    lets wall-clock deltas between two loop_n values cancel host/RPC
    overheads (this container has no NTFF profile path).

    ablate: perf-debug only — subset of {"qk", "act", "mask", "pv", "tail"}
    to skip emitting, isolating per-engine throughput on HW. Output is
    garbage when non-empty.
    """
    nsq = s // SQ
    nck = s // CK
    npair = nsq // 2
    ablate = set(ablate)

    nc = bacc.Bacc("TRN2", target_bir_lowering=False, debug=False)

    qt_d = nc.dram_tensor("qt", [hpc, D, s], f16, kind="ExternalInput")
    kt_d = nc.dram_tensor("kt", [hpc, D, s], f16, kind="ExternalInput")
    vp_d = nc.dram_tensor("vp", [hpc, CK, nck * VW], f16, kind="ExternalInput")
    mk_d = nc.dram_tensor("mk", [npair, CK, nck * PW], f16, kind="ExternalInput")
    o_d = nc.dram_tensor("o", [hpc, nsq, VW, SQ], f16, kind="ExternalOutput")

    with tile.TileContext(nc) as tc:
        # "nord" disables the race detector (no cross-engine semaphores);
        # other ablations keep it on unless their combo is inherently racy.
        if "nord" in ablate or "act" in ablate:
            tc.race_detector_enabled = False
        ablate.discard("nord")
        with (
            tc.tile_pool(name="heads", bufs=hpc) as head_pool,
            tc.tile_pool(name="mask", bufs=npair) as mask_pool,
            # Split pt into two SBUF regions to avoid the measured SBUF
            # conflict between ACT's exp writes and the PE's PV-rhs reads:
            # ACT writes ptA; DVE's mask/STT ops write pt2; PV reads pt2.
            tc.tile_pool(name="pta", bufs=2) as pta_pool,
            tc.tile_pool(name="pt2", bufs=2) as pt2_pool,
            tc.tile_pool(name="tail", bufs=2) as tail_pool,
            tc.tile_pool(name="qk_ps", bufs=QK_BUFS, space="PSUM") as qk_pool,
            tc.tile_pool(name="o_ps", bufs=2, space="PSUM") as o_pool,
        ):
            qt_t, kt_t, vp_t = [], [], []
            for h in range(hpc):
                q_t = head_pool.tile([128, s], f16, name=f"qt_sb{h}", tag="qt")
                k_t = head_pool.tile([128, s], f16, name=f"kt_sb{h}", tag="kt")
                v_t = head_pool.tile([CK, nck * VW], f16, name=f"vp_sb{h}", tag="vp")
                # Q^T/K^T live duplicated in both partition halves so the two
                # row-packed K=64 matmuls can run concurrently on the PE.
                nc.sync.dma_start(out=q_t[0:D, :], in_=qt_d[h, :, :])
                nc.sync.dma_start(out=q_t[D:128, :], in_=qt_d[h, :, :])
                nc.sync.dma_start(out=k_t[0:D, :], in_=kt_d[h, :, :])
                nc.sync.dma_start(out=k_t[D:128, :], in_=kt_d[h, :, :])
                nc.sync.dma_start(out=v_t[:, :], in_=vp_d[h, :, :])
                qt_t.append(q_t)
                kt_t.append(k_t)
                vp_t.append(v_t)

            # The whole mask fits in SBUF — load it once, outside any
            # measurement loop (saves 8MB of DMA per pass).
            mk_t = {}     # p -> mask tile [128, nck*PW] (chunk-major columns)
            for p in range(npair):
                mk = mask_pool.tile([CK, nck * PW], f16, name=f"mk_sb{p}",
                                    tag="mk")
                nc.sync.dma_start(out=mk[:, :], in_=mk_d[p, :, :])
                mk_t[p] = mk

            pta_t = {}    # pair-idx -> ACT exp out [128, nact*PW] fp16
            pt2_t = {}    # pair-idx -> PV-facing p^T [128, nck*PW] fp16
            o_ps = {}     # (pair-idx, u) -> PSUM accumulator [VW, SQ]

            qk_pos = [0]  # issue counter: row-half alternates per issue

            def emit_qk_chunk(j, pairs, c, kind):
                """One chunk's two QK matmuls (same K^T weights, query blocks
                sa and sb) + exp (ACT) or fused Schraudolph (DVE)."""
                h, p = pairs[j]
                qk = None
                if "qk" not in ablate:
                    qk = qk_pool.tile([128, PW], f32, name=f"qk_{j}_{c}",
                                      tag="qk")
                    bp = 64 * (qk_pos[0] % 2)  # row-half alternates per issued
                    # slot so consecutive weight loads pull ahead
                    qk_pos[0] += 1
                    for u in range(2):
                        sqb = 2 * p + u
                        nc.tensor.matmul(
                            qk[:, u * SQ:(u + 1) * SQ],
                            lhsT=kt_t[h][bp:bp + D, c * CK:(c + 1) * CK],
                            rhs=qt_t[h][bp:bp + D, sqb * SQ:(sqb + 1) * SQ],
                            start=True,
                            stop=True,
                            tile_position=(bp, 0),
                        )
                if "act" in ablate or pt2_t[j] is None:
                    return
                lo = c * PW
                hi = (c + 1) * PW
                act_in = qk[:, :] if qk is not None else mk_t[p][:, lo:hi]
                if kind == "act":
                    act_out = (act_in if "act_inplace" in ablate
                               else pta_t[j][:, lo:hi])
                    nc.scalar.activation(act_out, act_in, FT.Exp,
                                         scale=0.125)
                else:
                    nc.vector.scalar_tensor_tensor(
                        pt2_t[j].bitcast(u16)[:, lo:hi], act_in, A_SCH,
                        mk_t[p][:, lo:hi],
                        op0=mybir.AluOpType.mult, op1=mybir.AluOpType.add,
                    )

            def emit_mask(j, pairs, clo, chi):
                """Apply the 0/1 keep-mask to ACT-path chunk cols [clo, chi)
                of p^T in one fp16 2x-mode DVE pass."""
                if "mask" in ablate:
                    return
                h, p = pairs[j]
                clo, chi = min(clo, nact), min(chi, nact)
                if clo >= chi:
                    return
                lo, hi = clo * PW, chi * PW
                nc.vector.tensor_tensor(
                    pt2_t[j][:, lo:hi], pta_t[j][:, lo:hi],
                    mk_t[p][:, lo:hi],
                    op=mybir.AluOpType.mult,
                )

            def emit_pv(j, pairs, clo, chi):
                """PV matmuls for chunks [clo, chi): per chunk, both query
                blocks' matmuls share the V' weights."""
                if "pv" in ablate:
                    return
                pt = pt2_t[j]
                if pt is None:
                    pt = mk_t[pairs[j][1]]  # stand-in for PE-only ablations
                h, p = pairs[j]
                if "pvq" in ablate:
                    # rhs from a never-written tile (perf probe); wrap the
                    # column index into qt's [128, s] extent
                    qsrc = qt_t[h]
                    for c in range(clo, chi):
                        for u in range(2):
                            nc.tensor.matmul(
                                o_ps[(j, u)][:, :],
                                lhsT=vp_t[h][:, c * VW:c * VW + VW],
                                rhs=qsrc[:, (c % 2) * PW + u * SQ:
                                         (c % 2) * PW + (u + 1) * SQ],
                                start=True, stop=True, skip_group_check=True,
                            )
                    return
                for c in range(clo, chi):
                    for u in range(2):
                        nc.tensor.matmul(
                            o_ps[(j, u)][:, :],
                            lhsT=vp_t[h][:, c * VW:c * VW + VW],
                            rhs=pt[:, c * PW + u * SQ:c * PW + (u + 1) * SQ],
                            start=(c == 0 or "pvw" in ablate),
                            stop=(c == nck - 1 or "pvw" in ablate),
                            skip_group_check="pvw" in ablate,
                        )

            def emit_tail(j, pairs):
                """Evacuate O^T' (unnormalized + Z row) as fp16 and store."""
                if "tail" in ablate or "pv" in ablate:
                    return
                h, p = pairs[j]
                for u in range(2):
                    sqb = 2 * p + u
                    ot = tail_pool.tile([VW, SQ], f16, name=f"ot_{j}_{u}",
                                        tag="ot")
                    if tail_engine == "act":
                        nc.scalar.copy(ot[:, :], o_ps[(j, u)][:, :])
                    else:
                        nc.vector.tensor_copy(ot[:, :], o_ps[(j, u)][:, :])
                    nc.sync.dma_start(out=o_d[h, sqb, :, :], in_=ot[:, :])

            def emit_alloc(j):
                if not ({"act", "mask"} <= ablate):
                    pta_t[j] = pta_pool.tile(
                        [128, max(nact, 1) * PW], f16, name=f"pta_{j}",
                        tag="pta")
                    pt2_t[j] = pt2_pool.tile(
                        [128, nck * PW], f16, name=f"pt2_{j}", tag="pt2")
                else:
                    pta_t[j] = None
                    pt2_t[j] = None
                if "pv" not in ablate:
                    for u in range(2):
                        o_ps[(j, u)] = o_pool.tile(
                            [VW, SQ], f32, name=f"ops_{j}_{u}", tag="ops")

            def emit_pair(j, pairs):
                """Chunk-major emission for pair j, interleaved with pair
                j-1's masks + PV blocks.  The DVE chunks are spread through
                the middle of the pair with the previous pair's mask ops
                slotted between their STTs on the DVE queue, so every
                PSUM-slot heir is freed on time by a self-paced engine: no
                QK issue ever waits long on the other engine's backlog."""
                emit_alloc(j)
                prev = j - 1 if j >= 1 else None
                if nck != 16:   # small-s debug builds: simple sequential
                    for c in range(nck):
                        emit_qk_chunk(j, pairs, c, "act" if c < nact else "dve")
                    emit_mask(j, pairs, 0, nck)
                    if prev is not None:
                        emit_pv(prev, pairs, 0, nck)
                        emit_tail(prev, pairs)
                    return
                acts = list(range(nact))
                dves = list(range(nact, nck))
                if prev is not None:
                    emit_mask(prev, pairs, 0, 4)
                for c in acts[0:3]:
                    emit_qk_chunk(j, pairs, c, "act")
                if dves:
                    emit_qk_chunk(j, pairs, dves[0], "dve")
                if prev is not None:
                    emit_mask(prev, pairs, 4, 8)
                for c in acts[3:5]:
                    emit_qk_chunk(j, pairs, c, "act")
                if prev is not None:
                    emit_pv(prev, pairs, 0, 8)
                if len(dves) > 1:
                    emit_qk_chunk(j, pairs, dves[1], "dve")
                for c in acts[5:7]:
                    emit_qk_chunk(j, pairs, c, "act")
                if prev is not None:
                    emit_mask(prev, pairs, 8, nact)
                if len(dves) > 2:
                    emit_qk_chunk(j, pairs, dves[2], "dve")
                for c in acts[7:9]:
                    emit_qk_chunk(j, pairs, c, "act")
                if prev is not None:
                    emit_pv(prev, pairs, 8, nck)
                    emit_tail(prev, pairs)
                for c in dves[3:]:
                    emit_qk_chunk(j, pairs, c, "dve")
                for c in acts[9:]:
                    emit_qk_chunk(j, pairs, c, "act")

            def emit_drain(j, pairs):
                emit_mask(j, pairs, 0, nact)
                emit_pv(j, pairs, 0, nck)
                emit_tail(j, pairs)

            def emit_all():
                pairs = [(h, p) for h in range(hpc) for p in range(npair)]
                for j in range(len(pairs)):
                    emit_pair(j, pairs)
                if pairs:
                    emit_drain(len(pairs) - 1, pairs)

            if loop_n is None:
                emit_all()
            else:
                hints = (mybir.EngineType.PE, mybir.EngineType.Activation,
                         mybir.EngineType.DVE)
                with tc.For_i(0, loop_n, 1, hint_engines=hints,
                              staggered_reset=bool(loop_stagger)):
                    for _ in range(loop_body):
                        emit_all()

    nc.finalize()
    return nc


def shard_inputs(K, Q, V, mask, hpc=HPC, s=S, n_cores=N_CORES, nact=NACT):
    """Full inputs -> per-core in_maps with device-friendly host layouts."""
    nsq = s // SQ
    nck = s // CK
    npair = max(nsq // 2, 1)
    n_units = n_cores * hpc
    Kf = np.asarray(K, np.float32).reshape(n_units, s, D)
    Qf = np.asarray(Q, np.float32).reshape(n_units, s, D)
    Vf = np.asarray(V, np.float32).reshape(n_units, s, D)
    keepT = (~np.asarray(mask).reshape(s, s)).T  # [k, q], True = attend
    # ACT chunks: 0/1 multiplier.  DVE chunks: Schraudolph additive bias.
    nact = nact if nck == 16 else nck
    mk_f = keepT.astype(np.float16).reshape(nck, CK, s)
    dve = np.where(keepT.reshape(nck, CK, s)[nact:] > 0, np.float16(B_KEEP),
                   np.float16(B_MASK))
    mk_full = np.concatenate([mk_f[:nact], dve], axis=0)  # [nck, CK, s]
    # -> [npair, CK, nck * (2*SQ)]: chunk-major, the pair's two query
    # blocks (u = 0, 1) side by side within each chunk's column span.
    mk_host = np.ascontiguousarray(
        mk_full.reshape(nck, CK, npair, 2, SQ)
        .transpose(2, 1, 0, 3, 4)
        .reshape(npair, CK, nck * 2 * SQ)
    )
    in_maps = []
    for c in range(n_cores):
        sl = slice(c * hpc, (c + 1) * hpc)
        qt = np.ascontiguousarray(Qf[sl].transpose(0, 2, 1)).astype(np.float16)
        kt = np.ascontiguousarray(Kf[sl].transpose(0, 2, 1)).astype(np.float16)
        vp = np.zeros((hpc, s, VW), np.float16)
        vp[:, :, :D] = Vf[sl]
        vp[:, :, D] = 1.0
        vp = np.ascontiguousarray(
            vp.reshape(hpc, nck, CK, VW).transpose(0, 2, 1, 3)
            .reshape(hpc, CK, nck * VW)
        )
        in_maps.append({"qt": qt, "kt": kt, "vp": vp, "mk": mk_host})
    return in_maps


_NC_CACHE = {}


def _get_nc():
    if "nc" not in _NC_CACHE:
        _NC_CACHE["nc"] = build_nc()
    return _NC_CACHE["nc"]


def run_sharded(in_maps, trace=False, **kwargs):
    return run_bass_kernel_spmd(
        _get_nc(), in_maps, core_ids=list(range(N_CORES)), trace=trace, **kwargs
    )


def unshard_output(per_core_raw, hpc=HPC, s=S):
    """[hpc, nsq, VW, SQ] raw blocks per core -> [n*hpc, s, D] normalized.

    Row D of each block is the softmax denominator Z; dividing and
    transposing here is O(S*D) host work (same order as unsharding).
    """
    n = len(per_core_raw)
    out = np.empty((n * hpc, s, D), np.float32)
    for c, o in enumerate(per_core_raw):
        of = np.asarray(o, np.float32)              # raw blocks arrive fp16
        ot = of[:, :, :D, :] / of[:, :, D:D + 1, :]  # [hpc, nsq, D, SQ]
        out[c * hpc:(c + 1) * hpc] = (
            ot.transpose(0, 1, 3, 2).reshape(hpc, s, D))
    return out


def assemble_output(results):
    out = unshard_output([results[c]["o"] for c in range(N_CORES)])
    return out.reshape(B, H, S, D)


def kernel(K, Q, V, mask):
    in_maps = shard_inputs(K, Q, V, mask)
    res = run_sharded(in_maps)
    return assemble_output(res.results)
